# revision 1
# baseline (speedup 1.0000x reference)
"""DeepSeekMoE forward on 8 Trainium2 NeuronCores (Bass/Tile).

Strategy (expert-parallel, host dispatch/combine):
  - Router (sigmoid scores + top-4 + gating) computed on host with jax-CPU,
    bitwise-matching the reference's op sequence.
  - 24 uniform "FFN jobs": 16 routed experts (tokens gathered per expert,
    padded to capacity C) + 2 shared experts x 4 token-shards of 2048.
  - Each core runs 3 jobs: 1 shared-expert shard + its 2 routed experts.
    Per job: H^T = relu(W1^T X^T + b1); Y^T = W2^T H^T + b2, computed with
    feature-major fp32r matmuls (full-rate on trn2 PE for moving dim >=256).
  - Host scatters routed outputs back with gating weights and adds the
    residual + shared outputs.
"""

import numpy as np

D_MODEL, D_FF, NS, NR, KR = 2048, 1408, 2, 16, 4
P = 128
DT = D_MODEL // P  # 16
FT = D_FF // P     # 11
NCORES = 8
JOBS = 3           # per core: [shared shard, routed expert A, routed expert B]
SH_TOK = 2048      # shared-expert shard size (per core)

_prog_cache = {}
LAST_RESULT = None  # BassKernelResults of the most recent device run


def _ensure_ntff_hook():
    """This image's `antenv` lacks the `axon_hooks` get/set registry that
    `run_bass_kernel_spmd(trace=True)` imports under axon; install an
    equivalent shim backed by the libaxon ctypes profiler so tracing works
    (and BASS_TRACE=1 doesn't crash the run)."""
    try:
        from antenv.axon_hooks import get_axon_ntff_profile_hook  # noqa: F401
        return
    except ImportError:
        pass
    import sys
    import types
    try:
        import antenv
        mod = types.ModuleType("antenv.axon_hooks")
        _hook = [None]
        mod.set_axon_ntff_profile_hook = lambda h: _hook.__setitem__(0, h)
        mod.get_axon_ntff_profile_hook = lambda: _hook[0]
        sys.modules["antenv.axon_hooks"] = mod
        antenv.axon_hooks = mod
        from trn_agent_boot.trn_boot import _ntff_profile_via_ctypes
        mod.set_axon_ntff_profile_hook(
            _ntff_profile_via_ctypes("/opt/axon/libaxon_pjrt.so")
        )
    except Exception:
        pass


def _plan_chunks(block):
    """Split a block (multiple of 128, >=256) into moving-dim chunks in
    {256, 384, 512} so every fp32r matmul has moving dim >= 256."""
    n8 = block // P
    assert block % P == 0 and n8 >= 2
    out = []
    while n8 > 0:
        if n8 in (2, 3, 4):
            out.append(n8 * P)
            n8 = 0
        elif n8 == 5:
            out += [2 * P, 3 * P]
            n8 = 0
        else:
            out.append(4 * P)
            n8 -= 4
    return out


def _plan_blocks(C):
    """Split capacity C into token blocks of at most 1152 (SBUF budget),
    each a multiple of 128 and >= 256."""
    blocks = []
    rem = C
    while rem > 0:
        if rem <= 1152:
            blocks.append(rem)
            rem = 0
        elif rem - 1024 >= 256:
            blocks.append(1024)
            rem -= 1024
        else:
            b = (rem // 2 // P) * P
            blocks += [b, rem - b]
            rem = 0
    assert sum(blocks) == C and all(b >= 256 and b % P == 0 for b in blocks)
    return blocks


def _build_program(C):
    import concourse.mybir as mybir
    import concourse.tile as tile
    from concourse import bacc

    F32 = mybir.dt.float32
    F32R = mybir.dt.float32r
    Relu = mybir.ActivationFunctionType.Relu
    Identity = mybir.ActivationFunctionType.Identity

    job_tokens = [SH_TOK, C, C]
    job_blocks = [_plan_blocks(t) for t in job_tokens]

    nc = bacc.Bacc(None, target_bir_lowering=False)
    xt = nc.dram_tensor("xt", [JOBS, P, DT, C], F32R, kind="ExternalInput")
    w1 = nc.dram_tensor("w1", [JOBS, FT, P, DT, P], F32R, kind="ExternalInput")
    w2 = nc.dram_tensor("w2", [JOBS, DT, P, FT, P], F32R, kind="ExternalInput")
    b1 = nc.dram_tensor("b1", [P, JOBS * FT], F32, kind="ExternalInput")
    b2 = nc.dram_tensor("b2", [P, JOBS * DT], F32, kind="ExternalInput")
    yt = nc.dram_tensor("yt", [JOBS, DT, P, C], F32, kind="ExternalOutput")

    with tile.TileContext(nc) as tc:
        with (
            tc.tile_pool(name="const", bufs=1) as const,
            tc.tile_pool(name="x", bufs=1) as xpool,
            tc.tile_pool(name="h", bufs=1) as hpool,
            tc.tile_pool(name="w1p", bufs=3) as w1pool,
            tc.tile_pool(name="w2p", bufs=4) as w2pool,
            tc.tile_pool(name="y", bufs=3) as ypool,
            tc.tile_pool(name="ps", bufs=6, space="PSUM") as pspool,
        ):
            b1t = const.tile([P, JOBS * FT], F32)
            nc.gpsimd.dma_start(b1t[:], b1[:, :])
            b2t = const.tile([P, JOBS * DT], F32)
            nc.gpsimd.dma_start(b2t[:], b2[:, :])

            def emit_block(j, off, blk, xts, chunks):
                h_t = hpool.tile([P, FT, blk], F32R, tag="h")
                for ft in range(FT):
                    if ft == 0:
                        w1_t = w1_firsts.pop(0)
                    else:
                        w1_t = w1pool.tile([P, DT, P], F32R, tag="w1")
                        nc.sync.dma_start(w1_t[:], w1[j, ft])
                    coff = 0
                    for ch in chunks:
                        ps = pspool.tile([P, 512], F32, tag="ps")
                        for ko in range(DT):
                            lhsT = (
                                w1_t[ko][:]
                                if isinstance(w1_t, list)
                                else w1_t[:, ko]
                            )
                            nc.tensor.matmul(
                                ps[:, :ch],
                                lhsT,
                                xts[ko][:, coff : coff + ch],
                                start=(ko == 0),
                                stop=(ko == DT - 1),
                            )
                        nc.scalar.activation(
                            h_t[:, ft, coff : coff + ch],
                            ps[:, :ch],
                            Relu,
                            bias=b1t[:, j * FT + ft : j * FT + ft + 1],
                        )
                        coff += ch

                for dtile in range(DT):
                    w2_t = w2pool.tile([P, FT, P], F32R, tag="w2")
                    nc.sync.dma_start(w2_t[:], w2[j, dtile])
                    y_t = ypool.tile([P, 1152], F32, tag="y")
                    coff = 0
                    for ch in chunks:
                        ps = pspool.tile([P, 512], F32, tag="ps")
                        for ko in range(FT):
                            nc.tensor.matmul(
                                ps[:, :ch],
                                w2_t[:, ko],
                                h_t[:, ko, coff : coff + ch],
                                start=(ko == 0),
                                stop=(ko == FT - 1),
                            )
                        nc.scalar.activation(
                            y_t[:, coff : coff + ch],
                            ps[:, :ch],
                            Identity,
                            bias=b2t[:, j * DT + dtile : j * DT + dtile + 1],
                        )
                        coff += ch
                    # Y rides the ACT HW-DGE ring: keeps the SP ring free
                    # so the next block's X transfer isn't stuck behind
                    # sem-gated output writes (single-ring configs showed
                    # 10-16us PE gaps at every block boundary)
                    nc.scalar.dma_start(
                        yt[j, dtile, :, off : off + blk], y_t[:, :blk]
                    )

            w1_firsts = []

            def load_w1_first(j):
                t = w1pool.tile([P, DT, P], F32R, tag="w1")
                nc.sync.dma_start(t[:], w1[j, 0])
                w1_firsts.append(t)

            # Block 0 uses 16 separate per-ko tiles in a short-lived pool so
            # dependency tracking is per-ko (tile-granular): the first matmul
            # group starts as soon as x[ko=0] + w1[ft=0] land instead of
            # waiting ~24us for the whole 8MB block. The pool is exited
            # before the steady-state "x" pool allocates, so the regions
            # reuse the same SBUF.
            blk0 = job_blocks[0][0]
            with tc.tile_pool(name="x0", bufs=1) as x0pool:
                # per-ko weight tiles for the very first matmul group too, so
                # MM(ft0, ko0) waits on 64KB + 0.5MB instead of 1MB + 8MB
                w1k0 = []
                xts0 = []
                for ko in range(DT):
                    wk = x0pool.tile([P, P], F32R, tag=f"w1k{ko}")
                    nc.sync.dma_start(wk[:], w1[0, 0, :, ko, :])
                    w1k0.append(wk)
                    xk = x0pool.tile([P, blk0], F32R, tag=f"xk{ko}")
                    nc.sync.dma_start(xk[:], xt[0, :, ko, 0:blk0])
                    xts0.append(xk)
                w1_firsts.append(w1k0)
                emit_block(0, 0, blk0, xts0, _plan_chunks(blk0))

            with tc.tile_pool(name="x", bufs=1) as xpool:
                for j in range(JOBS):
                    off = blk0 if j == 0 else 0
                    for blk in job_blocks[j][(1 if j == 0 else 0):]:
                        chunks = _plan_chunks(blk)
                        load_w1_first(j)
                        # X rides the GPSIMD SWDGE ring: the SP ring's
                        # sequencer stalls on each pool-WAR-gated weight DMA,
                        # so anything queued behind the weights transfers at
                        # consumption pace. On its own ring the next block's X
                        # lands during the previous block's mm2 phase.
                        xt_t = xpool.tile([P, DT, blk], F32R, tag="x")
                        nc.gpsimd.dma_start(
                            xt_t[:], xt[j, :, :, off : off + blk]
                        )
                        emit_block(
                            j, off, blk,
                            [xt_t[:, ko] for ko in range(DT)], chunks,
                        )
                        off += blk

    nc.compile()
    return nc


def _routing(flat, centroids, bias):
    """Replicate the reference router bitwise: jax-CPU sigmoid scores,
    stable top-4 (argsort matches lax.top_k tie-breaking), normalized gates."""
    import jax
    import jax.numpy as jnp

    cpu = jax.devices("cpu")[0]
    with jax.default_device(cpu):
        scores = np.asarray(
            jax.nn.sigmoid(jnp.asarray(flat) @ jnp.asarray(centroids).T)
            + jnp.asarray(bias)
        )
    idx = np.argsort(-scores, axis=-1, kind="stable")[:, :KR]
    vals = np.take_along_axis(scores, idx, axis=-1)
    gating = vals / np.maximum(vals.sum(-1, keepdims=True, dtype=np.float32), 1e-8)
    return idx.astype(np.int32), gating.astype(np.float32)


def _feat_major(x_td):
    """[T, D] (rows=tokens) -> [P, D//P, T] feature-major device layout."""
    d = x_td.shape[1]
    return np.ascontiguousarray(x_td.T.reshape(d // P, P, -1).transpose(1, 0, 2))


def _w_tiles(w, kdim, mdim):
    """[K, M] -> [M//P, P(k_inner), K//P, P(m_inner)] lhsT tile layout."""
    kt, mt = kdim // P, mdim // P
    return np.ascontiguousarray(
        w.reshape(kt, P, mt, P).transpose(2, 1, 0, 3)
    )


def kernel(u, shared_w1, shared_b1, shared_w2, shared_b2,
           routed_w1, routed_b1, routed_w2, routed_b2, centroids, bias):
    from concourse.bass_utils import run_bass_kernel_spmd

    _ensure_ntff_hook()
    u = np.asarray(u, dtype=np.float32)
    b, s, d = u.shape
    flat = u.reshape(-1, d)
    T = flat.shape[0]

    idx, gating = _routing(flat, np.asarray(centroids, np.float32),
                           np.asarray(bias, np.float32))

    # per-expert token lists (ascending token id) and their gate values
    tok_lists, gate_lists = [], []
    for e in range(NR):
        hit = idx == e                        # [T, KR]
        rows = np.nonzero(hit.any(axis=1))[0]
        g = gating[hit].reshape(-1)           # row-major -> ascending token id
        tok_lists.append(rows)
        gate_lists.append(g.astype(np.float32))

    max_count = max(len(r) for r in tok_lists)
    C = max(256, -(-max_count // P) * P)
    key = (C,)
    if key not in _prog_cache:
        _prog_cache[key] = _build_program(C)
    nc = _prog_cache[key]

    sw1 = np.asarray(shared_w1, np.float32)
    sb1 = np.asarray(shared_b1, np.float32)
    sw2 = np.asarray(shared_w2, np.float32)
    sb2 = np.asarray(shared_b2, np.float32)
    rw1 = np.asarray(routed_w1, np.float32)
    rb1 = np.asarray(routed_b1, np.float32)
    rw2 = np.asarray(routed_w2, np.float32)
    rb2 = np.asarray(routed_b2, np.float32)

    rw1_t = [_w_tiles(rw1[e], D_MODEL, D_FF) for e in range(NR)]
    rw2_t = [_w_tiles(rw2[e], D_FF, D_MODEL) for e in range(NR)]
    sw1_t = [_w_tiles(sw1[n], D_MODEL, D_FF) for n in range(NS)]
    sw2_t = [_w_tiles(sw2[n], D_FF, D_MODEL) for n in range(NS)]

    in_maps = []
    for core in range(NCORES):
        sh_e = core % NS
        sh_off = (core // NS) * SH_TOK
        e0, e1 = 2 * core, 2 * core + 1

        xt = np.zeros((JOBS, P, DT, C), np.float32)
        xt[0, :, :, :SH_TOK] = _feat_major(flat[sh_off : sh_off + SH_TOK])
        for jslot, e in ((1, e0), (2, e1)):
            rows = tok_lists[e]
            if len(rows):
                xt[jslot, :, :, : len(rows)] = _feat_major(flat[rows])

        w1 = np.stack([sw1_t[sh_e], rw1_t[e0], rw1_t[e1]])
        w2 = np.stack([sw2_t[sh_e], rw2_t[e0], rw2_t[e1]])
        b1m = np.stack([sb1[sh_e], rb1[e0], rb1[e1]])   # [3, 1408]
        b2m = np.stack([sb2[sh_e], rb2[e0], rb2[e1]])   # [3, 2048]
        b1m = np.ascontiguousarray(b1m.reshape(JOBS * FT, P).T)  # [P, 33]
        b2m = np.ascontiguousarray(b2m.reshape(JOBS * DT, P).T)  # [P, 48]

        in_maps.append({"xt": xt, "w1": w1, "w2": w2, "b1": b1m, "b2": b2m})

    res = run_bass_kernel_spmd(nc, in_maps, core_ids=list(range(NCORES)))
    global LAST_RESULT
    LAST_RESULT = res

    out = flat.copy()
    for core in range(NCORES):
        ytc = res.results[core]["yt"]          # [JOBS, DT, P, C]
        sh_off = (core // NS) * SH_TOK
        out[sh_off : sh_off + SH_TOK] += (
            ytc[0].reshape(D_MODEL, C)[:, :SH_TOK].T
        )
        for jslot, e in ((1, 2 * core), (2, 2 * core + 1)):
            rows = tok_lists[e]
            if len(rows):
                ye = ytc[jslot].reshape(D_MODEL, C)[:, : len(rows)].T
                out[rows] += gate_lists[e][:, None] * ye

    return out.reshape(b, s, d)



# revision 4
# speedup vs baseline: 1.0955x; 1.0955x over previous
"""DeepSeekMoE forward on 8 Trainium2 NeuronCores (Bass/Tile).

Strategy (expert-parallel, host dispatch/combine):
  - Router (sigmoid scores + top-4 + gating) computed on host with jax-CPU,
    bitwise-matching the reference's op sequence.
  - 24 uniform "FFN jobs": 16 routed experts (tokens gathered per expert,
    padded per-slot) + 2 shared experts x 4 token-shards of 2048.
  - Each core runs 3 jobs: 1 shared-expert shard + 2 routed experts,
    paired largest-with-smallest so slot capacities are
    [2048, C0=pad(max count), C1=pad(9th-largest count)] instead of
    2x global max.
  - Per job: H^T = relu(W1^T X^T + b1); Y^T = W2^T H^T + b2, computed with
    feature-major bf16 matmuls (full-rate on trn2 PE, fp32 PSUM accum;
    bf16 also enables fast-weight-load so LDWEIGHTS hides under MMs).
  - Host scatters routed outputs back with gating weights and adds the
    residual + shared outputs.
"""

import numpy as np

D_MODEL, D_FF, NS, NR, KR = 2048, 1408, 2, 16, 4
P = 128
DT = D_MODEL // P  # 16
FT = D_FF // P     # 11
NCORES = 8
JOBS = 3           # per core: [shared shard, routed expert big, routed small]
SH_TOK = 2048      # shared-expert shard size (per core)

_prog_cache = {}
LAST_RESULT = None  # BassKernelResults of the most recent device run


def _ensure_ntff_hook():
    """This image's `antenv` lacks the `axon_hooks` get/set registry that
    `run_bass_kernel_spmd(trace=True)` imports under axon; install an
    equivalent shim backed by the libaxon ctypes profiler so tracing works
    (and BASS_TRACE=1 doesn't crash the run)."""
    try:
        from antenv.axon_hooks import get_axon_ntff_profile_hook  # noqa: F401
        return
    except ImportError:
        pass
    import sys
    import types
    try:
        import antenv
        mod = types.ModuleType("antenv.axon_hooks")
        _hook = [None]
        mod.set_axon_ntff_profile_hook = lambda h: _hook.__setitem__(0, h)
        mod.get_axon_ntff_profile_hook = lambda: _hook[0]
        sys.modules["antenv.axon_hooks"] = mod
        antenv.axon_hooks = mod
        from trn_agent_boot.trn_boot import _ntff_profile_via_ctypes
        mod.set_axon_ntff_profile_hook(
            _ntff_profile_via_ctypes("/opt/axon/libaxon_pjrt.so")
        )
    except Exception:
        pass


def _plan_chunks(block):
    """Split a block (multiple of 128) into moving-dim chunks in
    {128, 256, 384, 512} (PSUM bank is 512 fp32; bigger chunks amortize
    per-MM issue overhead)."""
    n8 = block // P
    assert block % P == 0 and n8 >= 1
    out = []
    while n8 > 0:
        if n8 in (1, 2, 3, 4):
            out.append(n8 * P)
            n8 = 0
        elif n8 == 5:
            out += [2 * P, 3 * P]
            n8 = 0
        else:
            out.append(4 * P)
            n8 -= 4
    return out


def _plan_blocks(C, first_small=False):
    """Split capacity C into token blocks of at most 1152 (SBUF budget),
    each a multiple of 128. first_small peels a 256-token block off the
    front so the pipeline primes with minimal DMA."""
    blocks = []
    rem = C
    if first_small and C > 512:
        blocks.append(256)
        rem -= 256
    while rem > 0:
        if rem <= 1152:
            blocks.append(rem)
            rem = 0
        elif rem - 1024 >= 256:
            blocks.append(1024)
            rem -= 1024
        else:
            b = (rem // 2 // P) * P
            blocks += [b, rem - b]
            rem = 0
    assert sum(blocks) == C and all(b >= P and b % P == 0 for b in blocks)
    return blocks


def _build_program(caps):
    import concourse.mybir as mybir
    import concourse.tile as tile
    from concourse import bacc

    F32 = mybir.dt.float32
    BF16 = mybir.dt.bfloat16
    Relu = mybir.ActivationFunctionType.Relu
    Identity = mybir.ActivationFunctionType.Identity

    job_tokens = list(caps)
    cmax = max(job_tokens)
    job_blocks = [
        _plan_blocks(t, first_small=(j == 0)) for j, t in enumerate(job_tokens)
    ]

    nc = bacc.Bacc(None, target_bir_lowering=False)
    xt = nc.dram_tensor("xt", [JOBS, P, DT, cmax], BF16, kind="ExternalInput")
    w1 = nc.dram_tensor("w1", [JOBS, FT, P, DT, P], BF16, kind="ExternalInput")
    w2 = nc.dram_tensor("w2", [JOBS, DT, P, FT, P], BF16, kind="ExternalInput")
    b1 = nc.dram_tensor("b1", [P, JOBS * FT], F32, kind="ExternalInput")
    b2 = nc.dram_tensor("b2", [P, JOBS * DT], F32, kind="ExternalInput")
    yt = nc.dram_tensor("yt", [JOBS, DT, P, cmax], BF16, kind="ExternalOutput")

    with tile.TileContext(nc) as tc:
        with (
            tc.tile_pool(name="const", bufs=1) as const,
            tc.tile_pool(name="h", bufs=1) as hpool,
            tc.tile_pool(name="w1p", bufs=4) as w1pool,
            tc.tile_pool(name="w2p", bufs=6) as w2pool,
            tc.tile_pool(name="y", bufs=4) as ypool,
            tc.tile_pool(name="ps", bufs=8, space="PSUM") as pspool,
        ):
            b1t = const.tile([P, JOBS * FT], F32)
            nc.gpsimd.dma_start(b1t[:], b1[:, :])
            b2t = const.tile([P, JOBS * DT], F32)
            nc.gpsimd.dma_start(b2t[:], b2[:, :])

            def emit_block(j, off, blk, xts, chunks):
                h_t = hpool.tile([P, FT, blk], BF16, tag="h")
                for ft in range(FT):
                    if ft == 0:
                        w1_t = w1_firsts.pop(0)
                    else:
                        w1_t = w1pool.tile([P, DT, P], BF16, tag="w1")
                        nc.sync.dma_start(w1_t[:], w1[j, ft])
                    coff = 0
                    for ch in chunks:
                        ps = pspool.tile([P, 512], F32, tag="ps")
                        for ko in range(DT):
                            lhsT = (
                                w1_t[ko][:]
                                if isinstance(w1_t, list)
                                else w1_t[:, ko]
                            )
                            nc.tensor.matmul(
                                ps[:, :ch],
                                lhsT,
                                xts[ko][:, coff : coff + ch],
                                start=(ko == 0),
                                stop=(ko == DT - 1),
                            )
                        nc.scalar.activation(
                            h_t[:, ft, coff : coff + ch],
                            ps[:, :ch],
                            Relu,
                            bias=b1t[:, j * FT + ft : j * FT + ft + 1],
                        )
                        coff += ch

                for dtile in range(DT):
                    w2_t = w2pool.tile([P, FT, P], BF16, tag="w2")
                    nc.sync.dma_start(w2_t[:], w2[j, dtile])
                    y_t = ypool.tile([P, 1152], BF16, tag="y")
                    coff = 0
                    for ch in chunks:
                        ps = pspool.tile([P, 512], F32, tag="ps")
                        for ko in range(FT):
                            nc.tensor.matmul(
                                ps[:, :ch],
                                w2_t[:, ko],
                                h_t[:, ko, coff : coff + ch],
                                start=(ko == 0),
                                stop=(ko == FT - 1),
                            )
                        nc.scalar.activation(
                            y_t[:, coff : coff + ch],
                            ps[:, :ch],
                            Identity,
                            bias=b2t[:, j * DT + dtile : j * DT + dtile + 1],
                        )
                        coff += ch
                    # Y rides the ACT HW-DGE ring: keeps the SP ring free
                    # for weight transfers.
                    nc.scalar.dma_start(
                        yt[j, dtile, :, off : off + blk], y_t[:, :blk]
                    )

            w1_firsts = []

            def load_w1_first(j):
                t = w1pool.tile([P, DT, P], BF16, tag="w1")
                nc.sync.dma_start(t[:], w1[j, 0])
                w1_firsts.append(t)

            # Block 0 uses per-ko tiles in a short-lived pool so dependency
            # tracking is tile-granular: the first matmul group starts as
            # soon as x[ko=0] + w1[ft=0,ko=0] land instead of waiting for
            # the whole block. The pool is exited before the steady-state
            # "x" pool allocates, so the regions reuse the same SBUF.
            blk0 = job_blocks[0][0]
            with tc.tile_pool(name="x0", bufs=1) as x0pool:
                w1k0 = []
                xts0 = []
                for ko in range(DT):
                    wk = x0pool.tile([P, P], BF16, tag=f"w1k{ko}")
                    nc.sync.dma_start(wk[:], w1[0, 0, :, ko, :])
                    w1k0.append(wk)
                    xk = x0pool.tile([P, blk0], BF16, tag=f"xk{ko}")
                    # ACT HW-DGE ring: idle at kernel start (Y comes much
                    # later), so the priming X tiles land at full speed.
                    nc.scalar.dma_start(xk[:], xt[0, :, ko, 0:blk0])
                    xts0.append(xk)
                w1_firsts.append(w1k0)
                emit_block(0, 0, blk0, xts0, _plan_chunks(blk0))

            # Steady state: X rides the GPSIMD SWDGE ring, double-buffered
            # (bufs=2) so block n+1's X starts landing while block n's mm1
            # still runs — a full block of slack vs the SW-DGE pace, and no
            # contention with weights (SP ring) or Y (ACT ring).
            with tc.tile_pool(name="x", bufs=2) as xpool:
                for j in range(JOBS):
                    off = blk0 if j == 0 else 0
                    for blk in job_blocks[j][(1 if j == 0 else 0):]:
                        chunks = _plan_chunks(blk)
                        load_w1_first(j)
                        xt_t = xpool.tile([P, DT, blk], BF16, tag="x")
                        nc.gpsimd.dma_start(
                            xt_t[:], xt[j, :, :, off : off + blk]
                        )
                        emit_block(
                            j, off, blk,
                            [xt_t[:, ko] for ko in range(DT)], chunks,
                        )
                        off += blk

    nc.compile()
    return nc


def _routing(flat, centroids, bias):
    """Replicate the reference router bitwise: jax-CPU sigmoid scores,
    stable top-4 (argsort matches lax.top_k tie-breaking), normalized gates."""
    import jax
    import jax.numpy as jnp

    cpu = jax.devices("cpu")[0]
    with jax.default_device(cpu):
        scores = np.asarray(
            jax.nn.sigmoid(jnp.asarray(flat) @ jnp.asarray(centroids).T)
            + jnp.asarray(bias)
        )
    idx = np.argsort(-scores, axis=-1, kind="stable")[:, :KR]
    vals = np.take_along_axis(scores, idx, axis=-1)
    gating = vals / np.maximum(vals.sum(-1, keepdims=True, dtype=np.float32), 1e-8)
    return idx.astype(np.int32), gating.astype(np.float32)


def _feat_major(x_td):
    """[T, D] (rows=tokens) -> [P, D//P, T] feature-major device layout."""
    d = x_td.shape[1]
    return np.ascontiguousarray(x_td.T.reshape(d // P, P, -1).transpose(1, 0, 2))


def _w_tiles(w, kdim, mdim):
    """[K, M] -> [M//P, P(k_inner), K//P, P(m_inner)] lhsT tile layout."""
    kt, mt = kdim // P, mdim // P
    return np.ascontiguousarray(
        w.reshape(kt, P, mt, P).transpose(2, 1, 0, 3)
    )


def kernel(u, shared_w1, shared_b1, shared_w2, shared_b2,
           routed_w1, routed_b1, routed_w2, routed_b2, centroids, bias):
    import ml_dtypes
    from concourse.bass_utils import run_bass_kernel_spmd

    BF16 = np.dtype(ml_dtypes.bfloat16)

    _ensure_ntff_hook()
    u = np.asarray(u, dtype=np.float32)
    b, s, d = u.shape
    flat = u.reshape(-1, d)
    T = flat.shape[0]

    idx, gating = _routing(flat, np.asarray(centroids, np.float32),
                           np.asarray(bias, np.float32))

    # per-expert token lists (ascending token id) and their gate values
    tok_lists, gate_lists = [], []
    for e in range(NR):
        hit = idx == e                        # [T, KR]
        rows = np.nonzero(hit.any(axis=1))[0]
        g = gating[hit].reshape(-1)           # row-major -> ascending token id
        tok_lists.append(rows)
        gate_lists.append(g.astype(np.float32))

    # Pair largest with smallest so slot capacities are
    # C0 = pad(count of largest), C1 = pad(9th-largest count).
    counts = np.array([len(r) for r in tok_lists])
    order = np.argsort(-counts, kind="stable")
    pad = lambda c: max(256, -(-c // P) * P)
    C0 = pad(counts[order[0]])
    C1 = pad(counts[order[NCORES]])
    caps = (SH_TOK, C0, C1)
    cmax = max(caps)

    if caps not in _prog_cache:
        _prog_cache[caps] = _build_program(caps)
    nc = _prog_cache[caps]

    flat_bf = flat.astype(BF16)
    sw1 = np.asarray(shared_w1, np.float32).astype(BF16)
    sb1 = np.asarray(shared_b1, np.float32)
    sw2 = np.asarray(shared_w2, np.float32).astype(BF16)
    sb2 = np.asarray(shared_b2, np.float32)
    rw1 = np.asarray(routed_w1, np.float32).astype(BF16)
    rb1 = np.asarray(routed_b1, np.float32)
    rw2 = np.asarray(routed_w2, np.float32).astype(BF16)
    rb2 = np.asarray(routed_b2, np.float32)

    rw1_t = [_w_tiles(rw1[e], D_MODEL, D_FF) for e in range(NR)]
    rw2_t = [_w_tiles(rw2[e], D_FF, D_MODEL) for e in range(NR)]
    sw1_t = [_w_tiles(sw1[n], D_MODEL, D_FF) for n in range(NS)]
    sw2_t = [_w_tiles(sw2[n], D_FF, D_MODEL) for n in range(NS)]

    in_maps = []
    core_experts = []
    for core in range(NCORES):
        sh_e = core % NS
        sh_off = (core // NS) * SH_TOK
        e0 = int(order[core])                 # big expert -> slot 1 (C0)
        e1 = int(order[2 * NCORES - 1 - core])  # small expert -> slot 2 (C1)
        core_experts.append((e0, e1))

        xt = np.zeros((JOBS, P, DT, cmax), BF16)
        xt[0, :, :, :SH_TOK] = _feat_major(flat_bf[sh_off : sh_off + SH_TOK])
        for jslot, e in ((1, e0), (2, e1)):
            rows = tok_lists[e]
            if len(rows):
                xt[jslot, :, :, : len(rows)] = _feat_major(flat_bf[rows])

        w1 = np.stack([sw1_t[sh_e], rw1_t[e0], rw1_t[e1]])
        w2 = np.stack([sw2_t[sh_e], rw2_t[e0], rw2_t[e1]])
        b1m = np.stack([sb1[sh_e], rb1[e0], rb1[e1]])   # [3, 1408]
        b2m = np.stack([sb2[sh_e], rb2[e0], rb2[e1]])   # [3, 2048]
        b1m = np.ascontiguousarray(b1m.reshape(JOBS * FT, P).T)  # [P, 33]
        b2m = np.ascontiguousarray(b2m.reshape(JOBS * DT, P).T)  # [P, 48]

        in_maps.append({"xt": xt, "w1": w1, "w2": w2, "b1": b1m, "b2": b2m})

    res = run_bass_kernel_spmd(nc, in_maps, core_ids=list(range(NCORES)))
    global LAST_RESULT
    LAST_RESULT = res

    out = flat.copy()
    for core in range(NCORES):
        ytc = res.results[core]["yt"]          # [JOBS, DT, P, cmax] bf16
        sh_off = (core // NS) * SH_TOK
        out[sh_off : sh_off + SH_TOK] += (
            ytc[0].reshape(D_MODEL, cmax)[:, :SH_TOK].T.astype(np.float32)
        )
        e0, e1 = core_experts[core]
        for jslot, e in ((1, e0), (2, e1)):
            rows = tok_lists[e]
            if len(rows):
                ye = ytc[jslot].reshape(D_MODEL, cmax)[:, : len(rows)].T
                out[rows] += gate_lists[e][:, None] * ye.astype(np.float32)

    return out.reshape(b, s, d)


# revision 7
# speedup vs baseline: 1.1174x; 1.0200x over previous
"""DeepSeekMoE forward on 8 Trainium2 NeuronCores (Bass/Tile).

Strategy (expert-parallel, host dispatch/combine):
  - Router (sigmoid scores + top-4 + gating) computed on host with jax-CPU,
    bitwise-matching the reference's op sequence.
  - 24 uniform "FFN jobs": 16 routed experts (tokens gathered per expert,
    padded per-slot) + 2 shared experts x 4 token-shards of 2048.
  - Each core runs 3 jobs: 1 shared-expert shard + 2 routed experts,
    paired largest-with-smallest so slot capacities are
    [2048, C0=pad(max count), C1=pad(9th-largest count)] instead of
    2x global max.
  - Per job: H^T = relu(W1^T X^T + b1); Y^T = W2^T H^T + b2, computed with
    feature-major bf16 matmuls (full-rate on trn2 PE, fp32 PSUM accum;
    bf16 also enables fast-weight-load so LDWEIGHTS hides under MMs).
  - Host scatters routed outputs back with gating weights and adds the
    residual + shared outputs.
"""

import numpy as np

D_MODEL, D_FF, NS, NR, KR = 2048, 1408, 2, 16, 4
P = 128
DT = D_MODEL // P  # 16
FT = D_FF // P     # 11
NCORES = 8
JOBS = 3           # per core: [shared shard, routed expert big, routed small]
SH_TOK = 2048      # shared-expert shard size (per core)

_prog_cache = {}
LAST_RESULT = None  # BassKernelResults of the most recent device run


def _ensure_ntff_hook():
    """This image's `antenv` lacks the `axon_hooks` get/set registry that
    `run_bass_kernel_spmd(trace=True)` imports under axon; install an
    equivalent shim backed by the libaxon ctypes profiler so tracing works
    (and BASS_TRACE=1 doesn't crash the run)."""
    try:
        from antenv.axon_hooks import get_axon_ntff_profile_hook  # noqa: F401
        return
    except ImportError:
        pass
    import sys
    import types
    try:
        import antenv
        mod = types.ModuleType("antenv.axon_hooks")
        _hook = [None]
        mod.set_axon_ntff_profile_hook = lambda h: _hook.__setitem__(0, h)
        mod.get_axon_ntff_profile_hook = lambda: _hook[0]
        sys.modules["antenv.axon_hooks"] = mod
        antenv.axon_hooks = mod
        from trn_agent_boot.trn_boot import _ntff_profile_via_ctypes
        mod.set_axon_ntff_profile_hook(
            _ntff_profile_via_ctypes("/opt/axon/libaxon_pjrt.so")
        )
    except Exception:
        pass


def _plan_chunks(block):
    """Split a block (multiple of 128) into moving-dim chunks in
    {128, 256, 384, 512} (PSUM bank is 512 fp32; bigger chunks amortize
    per-MM issue overhead)."""
    n8 = block // P
    assert block % P == 0 and n8 >= 1
    out = []
    while n8 > 0:
        if n8 in (1, 2, 3, 4):
            out.append(n8 * P)
            n8 = 0
        elif n8 == 5:
            out += [2 * P, 3 * P]
            n8 = 0
        else:
            out.append(4 * P)
            n8 -= 4
    return out


def _plan_blocks(C, first_small=False):
    """Split capacity C into token blocks of at most 1152 (SBUF budget),
    each a multiple of 128. first_small peels a 256-token block off the
    front so the pipeline primes with minimal DMA."""
    blocks = []
    rem = C
    if first_small and C > 768:
        # big enough that mm1 (one ring's worth of w1) covers the weight
        # DMA of the block, small enough to prime the pipeline fast
        blocks.append(512)
        rem -= 512
    while rem > 0:
        if rem <= 1152:
            blocks.append(rem)
            rem = 0
        elif rem - 1024 >= 256:
            blocks.append(1024)
            rem -= 1024
        else:
            b = (rem // 2 // P) * P
            blocks += [b, rem - b]
            rem = 0
    assert sum(blocks) == C and all(b >= P and b % P == 0 for b in blocks)
    return blocks


def _build_program(caps):
    import concourse.mybir as mybir
    import concourse.tile as tile
    from concourse import bacc

    F32 = mybir.dt.float32
    BF16 = mybir.dt.bfloat16
    Relu = mybir.ActivationFunctionType.Relu
    Identity = mybir.ActivationFunctionType.Identity

    job_tokens = list(caps)
    cmax = max(job_tokens)
    job_blocks = [
        _plan_blocks(t, first_small=(j == 0)) for j, t in enumerate(job_tokens)
    ]

    nc = bacc.Bacc(None, target_bir_lowering=False)
    xt = nc.dram_tensor("xt", [JOBS, P, DT, cmax], BF16, kind="ExternalInput")
    w1 = nc.dram_tensor("w1", [JOBS, FT, P, DT, P], BF16, kind="ExternalInput")
    w2 = nc.dram_tensor("w2", [JOBS, DT, P, FT, P], BF16, kind="ExternalInput")
    b1 = nc.dram_tensor("b1", [P, JOBS * FT], F32, kind="ExternalInput")
    b2 = nc.dram_tensor("b2", [P, JOBS * DT], F32, kind="ExternalInput")
    yt = nc.dram_tensor("yt", [JOBS, DT, P, cmax], BF16, kind="ExternalOutput")

    with tile.TileContext(nc) as tc:
        with (
            tc.tile_pool(name="const", bufs=1) as const,
            tc.tile_pool(name="h", bufs=1) as hpool,
            tc.tile_pool(name="w1p", bufs=4) as w1pool,
            tc.tile_pool(name="w2p", bufs=8) as w2pool,
            tc.tile_pool(name="y", bufs=4) as ypool,
            tc.tile_pool(name="ps", bufs=8, space="PSUM") as pspool,
        ):
            b1t = const.tile([P, JOBS * FT], F32)
            nc.gpsimd.dma_start(b1t[:], b1[:, :])
            b2t = const.tile([P, JOBS * DT], F32)
            nc.gpsimd.dma_start(b2t[:], b2[:, :])

            def emit_block(j, off, blk, xts, chunks):
                h_t = hpool.tile([P, FT, blk], BF16, tag="h")
                for ft in range(FT):
                    if ft == 0:
                        w1_t = w1_firsts.pop(0)
                    else:
                        w1_t = w1pool.tile([P, DT, P], BF16, tag="w1")
                        nc.sync.dma_start(w1_t[:], w1[j, ft])
                    coff = 0
                    for ch in chunks:
                        ps = pspool.tile([P, 512], F32, tag="ps")
                        for ko in range(DT):
                            lhsT = (
                                w1_t[ko][:]
                                if isinstance(w1_t, list)
                                else w1_t[:, ko]
                            )
                            nc.tensor.matmul(
                                ps[:, :ch],
                                lhsT,
                                xts[ko][:, coff : coff + ch],
                                start=(ko == 0),
                                stop=(ko == DT - 1),
                            )
                        nc.scalar.activation(
                            h_t[:, ft, coff : coff + ch],
                            ps[:, :ch],
                            Relu,
                            bias=b1t[:, j * FT + ft : j * FT + ft + 1],
                        )
                        coff += ch

                for dtile in range(DT):
                    w2_t = w2pool.tile([P, FT, P], BF16, tag="w2")
                    # ACT HW-DGE ring: splits weight bandwidth with the SP
                    # ring (w1) — the early blocks otherwise outrun a
                    # single ring and re-throttle HAM. Deep w2 pool keeps
                    # the WAR sem always-clear so the scalar FIFO never
                    # stalls behind this descriptor.
                    nc.scalar.dma_start(w2_t[:], w2[j, dtile])
                    y_t = ypool.tile([P, 1152], BF16, tag="y")
                    coff = 0
                    for ch in chunks:
                        ps = pspool.tile([P, 512], F32, tag="ps")
                        for ko in range(FT):
                            nc.tensor.matmul(
                                ps[:, :ch],
                                w2_t[:, ko],
                                h_t[:, ko, coff : coff + ch],
                                start=(ko == 0),
                                stop=(ko == FT - 1),
                            )
                        nc.scalar.activation(
                            y_t[:, coff : coff + ch],
                            ps[:, :ch],
                            Identity,
                            bias=b2t[:, j * DT + dtile : j * DT + dtile + 1],
                        )
                        coff += ch
                    # Y rides the ACT HW-DGE ring: keeps the SP ring free
                    # for weight transfers.
                    nc.scalar.dma_start(
                        yt[j, dtile, :, off : off + blk], y_t[:, :blk]
                    )

            w1_firsts = []

            def load_w1_first(j):
                t = w1pool.tile([P, DT, P], BF16, tag="w1")
                nc.sync.dma_start(t[:], w1[j, 0])
                w1_firsts.append(t)

            # Block 0 uses per-ko tiles in a short-lived pool so dependency
            # tracking is tile-granular: the first matmul group starts as
            # soon as x[ko=0] + w1[ft=0,ko=0] land instead of waiting for
            # the whole block. The pool is exited before the steady-state
            # "x" pool allocates, so the regions reuse the same SBUF.
            blk0 = job_blocks[0][0]
            with tc.tile_pool(name="x0", bufs=1) as x0pool:
                w1k0 = []
                xts0 = []
                for ko in range(DT):
                    wk = x0pool.tile([P, P], BF16, tag=f"w1k{ko}")
                    nc.sync.dma_start(wk[:], w1[0, 0, :, ko, :])
                    w1k0.append(wk)
                    xk = x0pool.tile([P, blk0], BF16, tag=f"xk{ko}")
                    # ACT HW-DGE ring: idle at kernel start (Y comes much
                    # later), so the priming X tiles land at full speed.
                    nc.scalar.dma_start(xk[:], xt[0, :, ko, 0:blk0])
                    xts0.append(xk)
                w1_firsts.append(w1k0)
                emit_block(0, 0, blk0, xts0, _plan_chunks(blk0))

            # Steady state: X rides the GPSIMD SWDGE ring, double-buffered
            # (bufs=2) so block n+1's X starts landing while block n's mm1
            # still runs — a full block of slack vs the SW-DGE pace, and no
            # contention with weights (SP ring) or Y (ACT ring).
            with tc.tile_pool(name="x", bufs=2) as xpool:
                for j in range(JOBS):
                    off = blk0 if j == 0 else 0
                    for blk in job_blocks[j][(1 if j == 0 else 0):]:
                        chunks = _plan_chunks(blk)
                        load_w1_first(j)
                        xt_t = xpool.tile([P, DT, blk], BF16, tag="x")
                        nc.gpsimd.dma_start(
                            xt_t[:], xt[j, :, :, off : off + blk]
                        )
                        emit_block(
                            j, off, blk,
                            [xt_t[:, ko] for ko in range(DT)], chunks,
                        )
                        off += blk

    nc.compile()
    return nc


def _routing(flat, centroids, bias):
    """Replicate the reference router bitwise: jax-CPU sigmoid scores,
    stable top-4 (argsort matches lax.top_k tie-breaking), normalized gates."""
    import jax
    import jax.numpy as jnp

    cpu = jax.devices("cpu")[0]
    with jax.default_device(cpu):
        scores = np.asarray(
            jax.nn.sigmoid(jnp.asarray(flat) @ jnp.asarray(centroids).T)
            + jnp.asarray(bias)
        )
    idx = np.argsort(-scores, axis=-1, kind="stable")[:, :KR]
    vals = np.take_along_axis(scores, idx, axis=-1)
    gating = vals / np.maximum(vals.sum(-1, keepdims=True, dtype=np.float32), 1e-8)
    return idx.astype(np.int32), gating.astype(np.float32)


def _feat_major(x_td):
    """[T, D] (rows=tokens) -> [P, D//P, T] feature-major device layout."""
    d = x_td.shape[1]
    return np.ascontiguousarray(x_td.T.reshape(d // P, P, -1).transpose(1, 0, 2))


def _w_tiles(w, kdim, mdim):
    """[K, M] -> [M//P, P(k_inner), K//P, P(m_inner)] lhsT tile layout."""
    kt, mt = kdim // P, mdim // P
    return np.ascontiguousarray(
        w.reshape(kt, P, mt, P).transpose(2, 1, 0, 3)
    )


def kernel(u, shared_w1, shared_b1, shared_w2, shared_b2,
           routed_w1, routed_b1, routed_w2, routed_b2, centroids, bias):
    import ml_dtypes
    from concourse.bass_utils import run_bass_kernel_spmd

    BF16 = np.dtype(ml_dtypes.bfloat16)

    _ensure_ntff_hook()
    u = np.asarray(u, dtype=np.float32)
    b, s, d = u.shape
    flat = u.reshape(-1, d)
    T = flat.shape[0]

    idx, gating = _routing(flat, np.asarray(centroids, np.float32),
                           np.asarray(bias, np.float32))

    # per-expert token lists (ascending token id) and their gate values
    tok_lists, gate_lists = [], []
    for e in range(NR):
        hit = idx == e                        # [T, KR]
        rows = np.nonzero(hit.any(axis=1))[0]
        g = gating[hit].reshape(-1)           # row-major -> ascending token id
        tok_lists.append(rows)
        gate_lists.append(g.astype(np.float32))

    # Pair largest with smallest so slot capacities are
    # C0 = pad(count of largest), C1 = pad(9th-largest count).
    counts = np.array([len(r) for r in tok_lists])
    order = np.argsort(-counts, kind="stable")
    pad = lambda c: max(256, -(-c // P) * P)
    C0 = pad(counts[order[0]])
    C1 = pad(counts[order[NCORES]])
    caps = (SH_TOK, C0, C1)
    cmax = max(caps)

    if caps not in _prog_cache:
        _prog_cache[caps] = _build_program(caps)
    nc = _prog_cache[caps]

    flat_bf = flat.astype(BF16)
    sw1 = np.asarray(shared_w1, np.float32).astype(BF16)
    sb1 = np.asarray(shared_b1, np.float32)
    sw2 = np.asarray(shared_w2, np.float32).astype(BF16)
    sb2 = np.asarray(shared_b2, np.float32)
    rw1 = np.asarray(routed_w1, np.float32).astype(BF16)
    rb1 = np.asarray(routed_b1, np.float32)
    rw2 = np.asarray(routed_w2, np.float32).astype(BF16)
    rb2 = np.asarray(routed_b2, np.float32)

    rw1_t = [_w_tiles(rw1[e], D_MODEL, D_FF) for e in range(NR)]
    rw2_t = [_w_tiles(rw2[e], D_FF, D_MODEL) for e in range(NR)]
    sw1_t = [_w_tiles(sw1[n], D_MODEL, D_FF) for n in range(NS)]
    sw2_t = [_w_tiles(sw2[n], D_FF, D_MODEL) for n in range(NS)]

    in_maps = []
    core_experts = []
    for core in range(NCORES):
        sh_e = core % NS
        sh_off = (core // NS) * SH_TOK
        e0 = int(order[core])                 # big expert -> slot 1 (C0)
        e1 = int(order[2 * NCORES - 1 - core])  # small expert -> slot 2 (C1)
        core_experts.append((e0, e1))

        xt = np.zeros((JOBS, P, DT, cmax), BF16)
        xt[0, :, :, :SH_TOK] = _feat_major(flat_bf[sh_off : sh_off + SH_TOK])
        for jslot, e in ((1, e0), (2, e1)):
            rows = tok_lists[e]
            if len(rows):
                xt[jslot, :, :, : len(rows)] = _feat_major(flat_bf[rows])

        w1 = np.stack([sw1_t[sh_e], rw1_t[e0], rw1_t[e1]])
        w2 = np.stack([sw2_t[sh_e], rw2_t[e0], rw2_t[e1]])
        b1m = np.stack([sb1[sh_e], rb1[e0], rb1[e1]])   # [3, 1408]
        b2m = np.stack([sb2[sh_e], rb2[e0], rb2[e1]])   # [3, 2048]
        b1m = np.ascontiguousarray(b1m.reshape(JOBS * FT, P).T)  # [P, 33]
        b2m = np.ascontiguousarray(b2m.reshape(JOBS * DT, P).T)  # [P, 48]

        in_maps.append({"xt": xt, "w1": w1, "w2": w2, "b1": b1m, "b2": b2m})

    res = run_bass_kernel_spmd(nc, in_maps, core_ids=list(range(NCORES)))
    global LAST_RESULT
    LAST_RESULT = res

    out = flat.copy()
    for core in range(NCORES):
        ytc = res.results[core]["yt"]          # [JOBS, DT, P, cmax] bf16
        sh_off = (core // NS) * SH_TOK
        out[sh_off : sh_off + SH_TOK] += (
            ytc[0].reshape(D_MODEL, cmax)[:, :SH_TOK].T.astype(np.float32)
        )
        e0, e1 = core_experts[core]
        for jslot, e in ((1, e0), (2, e1)):
            rows = tok_lists[e]
            if len(rows):
                ye = ytc[jslot].reshape(D_MODEL, cmax)[:, : len(rows)].T
                out[rows] += gate_lists[e][:, None] * ye.astype(np.float32)

    return out.reshape(b, s, d)


# revision 9
# speedup vs baseline: 1.1210x; 1.0032x over previous
"""DeepSeekMoE forward on 8 Trainium2 NeuronCores (Bass/Tile).

Strategy (expert-parallel, host dispatch/combine):
  - Router (sigmoid scores + top-4 + gating) computed on host with jax-CPU,
    bitwise-matching the reference's op sequence.
  - 24 uniform "FFN jobs": 16 routed experts (tokens gathered per expert,
    padded per-slot) + 2 shared experts x 4 token-shards of 2048.
  - Each core runs 3 jobs: 1 shared-expert shard + 2 routed experts,
    paired largest-with-smallest so slot capacities are
    [2048, C0=pad(max count), C1=pad(9th-largest count)] instead of
    2x global max.
  - Per job: H^T = relu(W1^T X^T + b1); Y^T = W2^T H^T + b2, computed with
    feature-major bf16 matmuls (full-rate on trn2 PE, fp32 PSUM accum;
    bf16 also enables fast-weight-load so LDWEIGHTS hides under MMs).
  - Host scatters routed outputs back with gating weights and adds the
    residual + shared outputs.
"""

import numpy as np

D_MODEL, D_FF, NS, NR, KR = 2048, 1408, 2, 16, 4
P = 128
DT = D_MODEL // P  # 16
FT = D_FF // P     # 11
NCORES = 8
JOBS = 3           # per core: [shared shard, routed expert big, routed small]
SH_TOK = 2048      # shared-expert shard size (per core)

_prog_cache = {}
LAST_RESULT = None  # BassKernelResults of the most recent device run


def _ensure_ntff_hook():
    """This image's `antenv` lacks the `axon_hooks` get/set registry that
    `run_bass_kernel_spmd(trace=True)` imports under axon; install an
    equivalent shim backed by the libaxon ctypes profiler so tracing works
    (and BASS_TRACE=1 doesn't crash the run)."""
    try:
        from antenv.axon_hooks import get_axon_ntff_profile_hook  # noqa: F401
        return
    except ImportError:
        pass
    import sys
    import types
    try:
        import antenv
        mod = types.ModuleType("antenv.axon_hooks")
        _hook = [None]
        mod.set_axon_ntff_profile_hook = lambda h: _hook.__setitem__(0, h)
        mod.get_axon_ntff_profile_hook = lambda: _hook[0]
        sys.modules["antenv.axon_hooks"] = mod
        antenv.axon_hooks = mod
        from trn_agent_boot.trn_boot import _ntff_profile_via_ctypes
        mod.set_axon_ntff_profile_hook(
            _ntff_profile_via_ctypes("/opt/axon/libaxon_pjrt.so")
        )
    except Exception:
        pass


def _plan_chunks(block):
    """Split a block (multiple of 128) into moving-dim chunks in
    {128, 256, 384, 512} (PSUM bank is 512 fp32; bigger chunks amortize
    per-MM issue overhead)."""
    n8 = block // P
    assert block % P == 0 and n8 >= 1
    out = []
    while n8 > 0:
        if n8 in (1, 2, 3, 4):
            out.append(n8 * P)
            n8 = 0
        elif n8 == 5:
            out += [2 * P, 3 * P]
            n8 = 0
        else:
            out.append(4 * P)
            n8 -= 4
    return out


def _plan_blocks(C, first_small=False):
    """Split capacity C into token blocks of at most 1152 (SBUF budget),
    each a multiple of 128. first_small peels a 256-token block off the
    front so the pipeline primes with minimal DMA."""
    blocks = []
    rem = C
    if first_small and C > 768:
        # big enough that mm1 (one ring's worth of w1) covers the weight
        # DMA of the block, small enough to prime the pipeline fast
        blocks.append(512)
        rem -= 512
    while rem > 0:
        if rem <= 1152:
            blocks.append(rem)
            rem = 0
        elif rem - 1024 >= 256:
            blocks.append(1024)
            rem -= 1024
        else:
            b = (rem // 2 // P) * P
            blocks += [b, rem - b]
            rem = 0
    assert sum(blocks) == C and all(b >= P and b % P == 0 for b in blocks)
    return blocks


def _build_program(caps):
    import concourse.mybir as mybir
    import concourse.tile as tile
    from concourse import bacc

    F32 = mybir.dt.float32
    BF16 = mybir.dt.bfloat16
    Relu = mybir.ActivationFunctionType.Relu
    Identity = mybir.ActivationFunctionType.Identity

    job_tokens = list(caps)
    cmax = max(job_tokens)
    job_blocks = [
        _plan_blocks(t, first_small=(j == 0)) for j, t in enumerate(job_tokens)
    ]

    nc = bacc.Bacc(None, target_bir_lowering=False)
    xt = nc.dram_tensor("xt", [JOBS, P, DT, cmax], BF16, kind="ExternalInput")
    w1 = nc.dram_tensor("w1", [JOBS, FT, P, DT, P], BF16, kind="ExternalInput")
    w2 = nc.dram_tensor("w2", [JOBS, DT, P, FT, P], BF16, kind="ExternalInput")
    b1 = nc.dram_tensor("b1", [P, JOBS * FT], F32, kind="ExternalInput")
    b2 = nc.dram_tensor("b2", [P, JOBS * DT], F32, kind="ExternalInput")
    yt = nc.dram_tensor("yt", [JOBS, DT, P, cmax], BF16, kind="ExternalOutput")

    with tile.TileContext(nc) as tc:
        with (
            tc.tile_pool(name="const", bufs=1) as const,
            tc.tile_pool(name="h", bufs=1) as hpool,
            tc.tile_pool(name="w1p", bufs=4) as w1pool,
            tc.tile_pool(name="w2p", bufs=8) as w2pool,
            tc.tile_pool(name="y", bufs=4) as ypool,
            tc.tile_pool(name="ps", bufs=8, space="PSUM") as pspool,
        ):
            b1t = const.tile([P, JOBS * FT], F32)
            nc.scalar.dma_start(b1t[:], b1[:, :])
            b2t = const.tile([P, JOBS * DT], F32)
            nc.scalar.dma_start(b2t[:], b2[:, :])

            # HAM warm-up: the first ~20us of real matmuls are DMA-paced
            # (priming), too sparse to flip the PE clock gate to 8/8 —
            # everything then runs at 1.2 GHz until ~3.4us of sustained
            # busy accumulates. Issue a burst of dummy matmuls with no DMA
            # dependency so the PE is densely busy from t~0: by the time
            # the first real matmul's operands land, the gate is at 8/8.
            warm = const.tile([P, 512], BF16)
            nc.vector.memset(warm[:], 0.0)
            warm_ps = pspool.tile([P, 512], F32, tag="ps")
            for _ in range(28):
                nc.tensor.matmul(
                    warm_ps[:], warm[:, :P], warm[:], start=True, stop=True
                )

            def emit_block(j, off, blk, xts, chunks):
                h_t = hpool.tile([P, FT, blk], BF16, tag="h")
                for ft in range(FT):
                    if ft == 0:
                        w1_t = w1_firsts.pop(0)
                    else:
                        w1_t = w1pool.tile([P, DT, P], BF16, tag="w1")
                        nc.sync.dma_start(w1_t[:], w1[j, ft])
                    coff = 0
                    for ch in chunks:
                        ps = pspool.tile([P, 512], F32, tag="ps")
                        for ko in range(DT):
                            lhsT = (
                                w1_t[ko][:]
                                if isinstance(w1_t, list)
                                else w1_t[:, ko]
                            )
                            nc.tensor.matmul(
                                ps[:, :ch],
                                lhsT,
                                xts[ko][:, coff : coff + ch],
                                start=(ko == 0),
                                stop=(ko == DT - 1),
                            )
                        nc.scalar.activation(
                            h_t[:, ft, coff : coff + ch],
                            ps[:, :ch],
                            Relu,
                            bias=b1t[:, j * FT + ft : j * FT + ft + 1],
                        )
                        coff += ch

                for dtile in range(DT):
                    w2_t = w2pool.tile([P, FT, P], BF16, tag="w2")
                    # ACT HW-DGE ring: splits weight bandwidth with the SP
                    # ring (w1) — the early blocks otherwise outrun a
                    # single ring and re-throttle HAM. Deep w2 pool keeps
                    # the WAR sem always-clear so the scalar FIFO never
                    # stalls behind this descriptor.
                    nc.scalar.dma_start(w2_t[:], w2[j, dtile])
                    y_t = ypool.tile([P, 1152], BF16, tag="y")
                    coff = 0
                    for ch in chunks:
                        ps = pspool.tile([P, 512], F32, tag="ps")
                        for ko in range(FT):
                            nc.tensor.matmul(
                                ps[:, :ch],
                                w2_t[:, ko],
                                h_t[:, ko, coff : coff + ch],
                                start=(ko == 0),
                                stop=(ko == FT - 1),
                            )
                        nc.scalar.activation(
                            y_t[:, coff : coff + ch],
                            ps[:, :ch],
                            Identity,
                            bias=b2t[:, j * DT + dtile : j * DT + dtile + 1],
                        )
                        coff += ch
                    # Y rides the ACT HW-DGE ring: keeps the SP ring free
                    # for weight transfers.
                    nc.scalar.dma_start(
                        yt[j, dtile, :, off : off + blk], y_t[:, :blk]
                    )

            w1_firsts = []

            def load_w1_first(j):
                t = w1pool.tile([P, DT, P], BF16, tag="w1")
                nc.sync.dma_start(t[:], w1[j, 0])
                w1_firsts.append(t)

            # Block 0 uses per-ko tiles in a short-lived pool so dependency
            # tracking is tile-granular: the first matmul group starts as
            # soon as x[ko=0] + w1[ft=0,ko=0] land instead of waiting for
            # the whole block. The pool is exited before the steady-state
            # "x" pool allocates, so the regions reuse the same SBUF.
            blk0 = job_blocks[0][0]
            with tc.tile_pool(name="x0", bufs=1) as x0pool:
                w1k0 = []
                xts0 = []
                for ko in range(DT):
                    wk = x0pool.tile([P, P], BF16, tag=f"w1k{ko}")
                    nc.sync.dma_start(wk[:], w1[0, 0, :, ko, :])
                    w1k0.append(wk)
                    xk = x0pool.tile([P, blk0], BF16, tag=f"xk{ko}")
                    # ACT HW-DGE ring: idle at kernel start (Y comes much
                    # later), so the priming X tiles land at full speed.
                    nc.scalar.dma_start(xk[:], xt[0, :, ko, 0:blk0])
                    xts0.append(xk)
                w1_firsts.append(w1k0)
                emit_block(0, 0, blk0, xts0, _plan_chunks(blk0))

            # Steady state: X rides the SP ring, double-buffered (bufs=2)
            # and issued before the block's w1 loads, so it sits at most
            # behind the previous block's consumption-paced w1 tiles —
            # ~a full block of slack. (The GPSIMD SW-DGE alternative
            # moves X at only ~110 GB/s and showed 35us late-landing X.)
            with tc.tile_pool(name="x", bufs=2) as xpool:
                for j in range(JOBS):
                    off = blk0 if j == 0 else 0
                    for blk in job_blocks[j][(1 if j == 0 else 0):]:
                        chunks = _plan_chunks(blk)
                        xt_t = xpool.tile([P, DT, blk], BF16, tag="x")
                        nc.sync.dma_start(
                            xt_t[:], xt[j, :, :, off : off + blk]
                        )
                        load_w1_first(j)
                        emit_block(
                            j, off, blk,
                            [xt_t[:, ko] for ko in range(DT)], chunks,
                        )
                        off += blk

    nc.compile()
    return nc


def _routing(flat, centroids, bias):
    """Replicate the reference router bitwise: jax-CPU sigmoid scores,
    stable top-4 (argsort matches lax.top_k tie-breaking), normalized gates."""
    import jax
    import jax.numpy as jnp

    cpu = jax.devices("cpu")[0]
    with jax.default_device(cpu):
        scores = np.asarray(
            jax.nn.sigmoid(jnp.asarray(flat) @ jnp.asarray(centroids).T)
            + jnp.asarray(bias)
        )
    idx = np.argsort(-scores, axis=-1, kind="stable")[:, :KR]
    vals = np.take_along_axis(scores, idx, axis=-1)
    gating = vals / np.maximum(vals.sum(-1, keepdims=True, dtype=np.float32), 1e-8)
    return idx.astype(np.int32), gating.astype(np.float32)


def _feat_major(x_td):
    """[T, D] (rows=tokens) -> [P, D//P, T] feature-major device layout."""
    d = x_td.shape[1]
    return np.ascontiguousarray(x_td.T.reshape(d // P, P, -1).transpose(1, 0, 2))


def _w_tiles(w, kdim, mdim):
    """[K, M] -> [M//P, P(k_inner), K//P, P(m_inner)] lhsT tile layout."""
    kt, mt = kdim // P, mdim // P
    return np.ascontiguousarray(
        w.reshape(kt, P, mt, P).transpose(2, 1, 0, 3)
    )


def kernel(u, shared_w1, shared_b1, shared_w2, shared_b2,
           routed_w1, routed_b1, routed_w2, routed_b2, centroids, bias):
    import ml_dtypes
    from concourse.bass_utils import run_bass_kernel_spmd

    BF16 = np.dtype(ml_dtypes.bfloat16)

    _ensure_ntff_hook()
    u = np.asarray(u, dtype=np.float32)
    b, s, d = u.shape
    flat = u.reshape(-1, d)
    T = flat.shape[0]

    idx, gating = _routing(flat, np.asarray(centroids, np.float32),
                           np.asarray(bias, np.float32))

    # per-expert token lists (ascending token id) and their gate values
    tok_lists, gate_lists = [], []
    for e in range(NR):
        hit = idx == e                        # [T, KR]
        rows = np.nonzero(hit.any(axis=1))[0]
        g = gating[hit].reshape(-1)           # row-major -> ascending token id
        tok_lists.append(rows)
        gate_lists.append(g.astype(np.float32))

    # Pair largest with smallest so slot capacities are
    # C0 = pad(count of largest), C1 = pad(9th-largest count).
    counts = np.array([len(r) for r in tok_lists])
    order = np.argsort(-counts, kind="stable")
    pad = lambda c: max(256, -(-c // P) * P)
    C0 = pad(counts[order[0]])
    C1 = pad(counts[order[NCORES]])
    caps = (SH_TOK, C0, C1)
    cmax = max(caps)

    if caps not in _prog_cache:
        _prog_cache[caps] = _build_program(caps)
    nc = _prog_cache[caps]

    flat_bf = flat.astype(BF16)
    sw1 = np.asarray(shared_w1, np.float32).astype(BF16)
    sb1 = np.asarray(shared_b1, np.float32)
    sw2 = np.asarray(shared_w2, np.float32).astype(BF16)
    sb2 = np.asarray(shared_b2, np.float32)
    rw1 = np.asarray(routed_w1, np.float32).astype(BF16)
    rb1 = np.asarray(routed_b1, np.float32)
    rw2 = np.asarray(routed_w2, np.float32).astype(BF16)
    rb2 = np.asarray(routed_b2, np.float32)

    rw1_t = [_w_tiles(rw1[e], D_MODEL, D_FF) for e in range(NR)]
    rw2_t = [_w_tiles(rw2[e], D_FF, D_MODEL) for e in range(NR)]
    sw1_t = [_w_tiles(sw1[n], D_MODEL, D_FF) for n in range(NS)]
    sw2_t = [_w_tiles(sw2[n], D_FF, D_MODEL) for n in range(NS)]

    in_maps = []
    core_experts = []
    for core in range(NCORES):
        sh_e = core % NS
        sh_off = (core // NS) * SH_TOK
        e0 = int(order[core])                 # big expert -> slot 1 (C0)
        e1 = int(order[2 * NCORES - 1 - core])  # small expert -> slot 2 (C1)
        core_experts.append((e0, e1))

        xt = np.zeros((JOBS, P, DT, cmax), BF16)
        xt[0, :, :, :SH_TOK] = _feat_major(flat_bf[sh_off : sh_off + SH_TOK])
        for jslot, e in ((1, e0), (2, e1)):
            rows = tok_lists[e]
            if len(rows):
                xt[jslot, :, :, : len(rows)] = _feat_major(flat_bf[rows])

        w1 = np.stack([sw1_t[sh_e], rw1_t[e0], rw1_t[e1]])
        w2 = np.stack([sw2_t[sh_e], rw2_t[e0], rw2_t[e1]])
        b1m = np.stack([sb1[sh_e], rb1[e0], rb1[e1]])   # [3, 1408]
        b2m = np.stack([sb2[sh_e], rb2[e0], rb2[e1]])   # [3, 2048]
        b1m = np.ascontiguousarray(b1m.reshape(JOBS * FT, P).T)  # [P, 33]
        b2m = np.ascontiguousarray(b2m.reshape(JOBS * DT, P).T)  # [P, 48]

        in_maps.append({"xt": xt, "w1": w1, "w2": w2, "b1": b1m, "b2": b2m})

    res = run_bass_kernel_spmd(nc, in_maps, core_ids=list(range(NCORES)))
    global LAST_RESULT
    LAST_RESULT = res

    out = flat.copy()
    for core in range(NCORES):
        ytc = res.results[core]["yt"]          # [JOBS, DT, P, cmax] bf16
        sh_off = (core // NS) * SH_TOK
        out[sh_off : sh_off + SH_TOK] += (
            ytc[0].reshape(D_MODEL, cmax)[:, :SH_TOK].T.astype(np.float32)
        )
        e0, e1 = core_experts[core]
        for jslot, e in ((1, e0), (2, e1)):
            rows = tok_lists[e]
            if len(rows):
                ye = ytc[jslot].reshape(D_MODEL, cmax)[:, : len(rows)].T
                out[rows] += gate_lists[e][:, None] * ye.astype(np.float32)

    return out.reshape(b, s, d)
